# revision 26
# baseline (speedup 1.0000x reference)
"""Causal self-attention Bass kernel for 8 trn2 NeuronCores.

Problem: B=4, T=2048, D=1024, H=16 causal self-attention (qkv proj + attn + out proj).

Sharding: core c = 2*b + g handles batch b (=c//2) and head-group g (=c%2, 8 heads).
Per core:
  - qkv projection column-shard: q,k,v columns for its 8 heads only.
  - flash-style attention in transposed-score layout sT[tk, tq]; softmax denominator
    via an extra ones-column in the AV matmul (row 64 of the [65, 512] psum output).
  - output projection row-shard (w_proj rows for its head dims) -> partial [T, D].
  - pairwise ReduceScatter {2b, 2b+1} sums the two head-group partials; chunked
    over 4 query blocks so the collective overlaps the attention/projection
    stream.  Even core ends with global rows {512J..512J+256}, odd core with
    {512J+256..512J+512}; host reassembles.

Pipeline: phase 2 runs query-block-outer (J = 0..3).  Step J computes attention
for all 4 head-pairs at query block J, while interleaving (as PE filler work
between ACT-gated attention iterations) the qk-projection for column block J+1
and the output projection + ReduceScatter chunk for block J-1.  The AV matmul is
software-pipelined one iteration behind the score matmul so the PE never waits
on the exp/mask chain.

Precision: bf16 operands everywhere on the matmul paths (1 cyc/row streaming,
FWL weight loads); psum accumulation stays f32.  The ReduceScatter runs in bf16;
host converts the output to f32.  b_v is folded into beta = b_proj(once per
pair) + w_proj_shard.T @ b_v_shard since softmax rows sum to 1.

Diagonal blocks: partial exp (cols >= 128*i only) and ONE strided triangle-mask
multiply covering both heads; AV matmuls read only the valid column range, so no
zero-fill of the masked region is needed.
"""

from contextlib import ExitStack

import ml_dtypes
import numpy as np

import concourse.bass as bass
import concourse.mybir as mybir
import concourse.tile as tile
from concourse import bacc
from concourse.bass_utils import run_bass_kernel_spmd

B, T, D, H = 4, 2048, 1024, 16
HD = D // H  # 64
NCORES = 8
P = 128
f32 = mybir.dt.float32
f32r = mybir.dt.float32r
bf16 = mybir.dt.bfloat16
EXP = mybir.ActivationFunctionType.Exp

_CACHE = {}
LAST_RESULTS = None
_DEBUG_SINK = None

# ReduceScatter chunk boundaries (input row ranges of the [T, D] partial sums)
CHUNKS = [(0, 512), (512, 1024), (1024, 1536), (1536, 2048)]


def _dbg(nc, name, ap):
    if _DEBUG_SINK is not None and name in _DEBUG_SINK:
        nc.sync.dma_start(_DEBUG_SINK[name].ap(), ap)


def _emit(nc, tc, x_d, wqk_d, wv_d, bqk_d, wproj_d, beta_d, out_d):
    with ExitStack() as ctx:
        # ---------------- constants / persistent tiles ----------------
        const = ctx.enter_context(tc.tile_pool(name="const", bufs=1))
        bootc = ctx.enter_context(tc.tile_pool(name="boot", bufs=1))
        ident_f = bootc.tile([P, P], bf16, tag="ident_f")
        nc.gpsimd.memset(ident_f[:], 0.0)
        nc.gpsimd.affine_select(
            out=ident_f[:], in_=ident_f[:],
            compare_op=mybir.AluOpType.not_equal, fill=1.0,
            base=0, pattern=[[-1, P]], channel_multiplier=1,
        )
        # triangle mask [128,256] = two copies of (keep iff f >= p)
        mask_tri2 = const.tile([P, 2 * P], bf16, tag="mask_tri2")
        nc.gpsimd.memset(mask_tri2[:], 1.0)
        for h0 in (0, P):
            nc.gpsimd.affine_select(
                out=mask_tri2[:, h0 : h0 + P], in_=mask_tri2[:, h0 : h0 + P],
                compare_op=mybir.AluOpType.is_ge, fill=0.0,
                base=0, pattern=[[1, P]], channel_multiplier=-1,
            )
        bq8 = const.tile([P, 8], f32, tag="bq8")
        beta_b = const.tile([P, D], bf16, tag="beta_b")

        def _load_small_consts():
            nc.sync.dma_start(bq8[:], bqk_d.ap().rearrange("m p one -> p (m one)"))
            nc.sync.dma_start(beta_b[0:1, :], beta_d.ap())
            nc.gpsimd.partition_broadcast(beta_b[:], beta_b[0:1, :], channels=P)
        # weight pools reserved here; DMAs are emitted after phase 1 starts
        # so the x loads win the DMA queue.
        wpp = ctx.enter_context(tc.tile_pool(name="wpp", bufs=1))
        wproj_t = [wpp.tile([P, D], bf16, tag=f"wp{hp}", name=f"wp{hp}") for hp in range(4)]
        wqkp = ctx.enter_context(tc.tile_pool(name="wqk", bufs=1))
        # one [P, 1024] tile per k-chunk holding all 8 m column blocks
        wq8 = [wqkp.tile([P, 1024], bf16, tag=f"wq8k{k}", name=f"wq8k{k}") for k in range(8)]
        _dbg(nc, "beta_b", beta_b[:])

        # persistent activations
        xt_pool = ctx.enter_context(tc.tile_pool(name="xt", bufs=1))
        xT = [xt_pool.tile([P, T], bf16, tag=f"xT{k}", name=f"xT{k}") for k in range(8)]
        vv_pool = ctx.enter_context(tc.tile_pool(name="vv", bufs=1))
        vv = [vv_pool.tile([P, 520], bf16, tag=f"vv{i}", name=f"vv{i}") for i in range(16)]
        on_pool = ctx.enter_context(tc.tile_pool(name="outn", bufs=1))
        outN = [[on_pool.tile([P, 512], bf16, tag=f"outN{mp}J{J}", name=f"outN{mp}J{J}")
                 for J in range(4)] for mp in range(4)]
        ones8 = const.tile([P, 8], bf16, tag="ones8")
        nc.vector.memset(ones8[:], 1.0)
        ones_src = ones8[:].rearrange("p (mp h one) -> p mp h one", mp=4, h=2)
        for i in range(16):
            dst = vv[i][:].rearrange("p (mp h d) -> p mp h d", mp=4, h=2)
            nc.vector.tensor_copy(dst[:, :, :, 64:65], ones_src[:, :, :, :])

        dram = ctx.enter_context(tc.tile_pool(name="dram", bufs=1, space="DRAM"))
        rs_in = dram.tile([T, D], bf16)
        rs_out = dram.tile([T // 2, D], bf16)
        dram2 = ctx.enter_context(tc.tile_pool(name="dram2", bufs=2, space="DRAM"))

        # ------- unified pipeline: x/v prep is filler work inside the steps -------
        with ExitStack() as p2:
            xload = p2.enter_context(tc.tile_pool(name="xload", bufs=3))
            wvp = p2.enter_context(tc.tile_pool(name="wv", bufs=1))
            qkt_pool = p2.enter_context(tc.tile_pool(name="qkt", bufs=1))
            qkT = [qkt_pool.tile([P, T], bf16, tag=f"qkT{m}", name=f"qkT{m}") for m in range(8)]
            atp = p2.enter_context(tc.tile_pool(name="atp", bufs=3))
            recip = p2.enter_context(tc.tile_pool(name="recip", bufs=2))
            bcast = p2.enter_context(tc.tile_pool(name="bcast", bufs=2))
            tmpb = p2.enter_context(tc.tile_pool(name="tmpb", bufs=4))
            qkps = p2.enter_context(tc.tile_pool(name="qkps", bufs=2, space="PSUM"))
            stps = p2.enter_context(tc.tile_pool(name="stps", bufs=2, space="PSUM"))
            oups = p2.enter_context(tc.tile_pool(name="oups", bufs=1, space="PSUM"))
            # [P, 1024] tiles, each holding two k-chunks side by side
            wv_t = [wvp.tile([P, 1024], bf16, tag=f"wvt{k2}", name=f"wvt{k2}") for k2 in range(4)]

            def x_loads(qq):
                # one [P, 2048] load covers two t-tiles (rows r0..r0+256)
                xi2 = []
                for ii in range(2):
                    xt_ = xload.tile([P, 2 * D], bf16, tag="x")
                    r0 = (qq * 4 + 2 * ii) * P
                    nc.sync.dma_start(
                        xt_[:].rearrange("p (b c) -> p b c", b=2),
                        x_d.ap()[r0 : r0 + 2 * P, :].rearrange("(b p) c -> p b c", p=P),
                    )
                    xi2.append(xt_)
                return [
                    xi2[ii // 2][:, (ii % 2) * D : (ii % 2 + 1) * D]
                    for ii in range(4)
                ]

            def trans_group(qq, k, xi):
                # transpose one k-chunk of a t-quarter into xT (evict on DVE)
                tp = qkps.tile([P, 512], bf16, tag="qkp", name="tp")
                for ii in range(4):
                    nc.tensor.transpose(
                        tp[:, ii * P : (ii + 1) * P],
                        xi[ii][:, k * P : (k + 1) * P],
                        ident_f[:],
                    )
                nc.vector.tensor_copy(xT[k][:, qq * 512 : (qq + 1) * 512], tp[:])

            def vproj_group(qq, il):
                i = qq * 4 + il
                ps = qkps.tile([P, 512], f32, tag="qkp", name="vp")
                for k in range(8):
                    nc.tensor.matmul(
                        ps[:],
                        xT[k][:, i * P : (i + 1) * P],
                        wv_t[k // 2][:, (k % 2) * 512 : (k % 2 + 1) * 512],
                        start=(k == 0), stop=(k == 7),
                    )
                # strided evict: psum [p, (mp h d)] d=64 -> vv [p, (mp h d65)]
                src = ps[:].rearrange("p (mp h d) -> p mp h d", mp=4, h=2)
                dst = vv[i][:].rearrange("p (mp h d) -> p mp h d", mp=4, h=2)
                nc.vector.tensor_copy(dst[:, :, :, 0:64], src[:, :, :, :])

            def qkproj_group(m, n):
                ps = qkps.tile([P, 512], f32, tag="qkp")
                for k in range(8):
                    nc.tensor.matmul(
                        ps[:], wq8[k][:, m * P : (m + 1) * P],
                        xT[k][:, n * 512 : (n + 1) * 512],
                        start=(k == 0), stop=(k == 7),
                    )
                nc.vector.tensor_scalar_add(
                    qkT[m][:, n * 512 : (n + 1) * 512], ps[:], bq8[:, m : m + 1]
                )

            def outproj_group(J, g):
                i = 4 * J + g // 2
                n = g % 2
                ps = qkps.tile([P, 512], f32, tag="qkp", name="fp")
                for hp in range(4):
                    nc.tensor.matmul(
                        ps[:],
                        outN[hp][J][:, (i % 4) * P : (i % 4 + 1) * P],
                        wproj_t[hp][:, n * 512 : (n + 1) * 512],
                        start=(hp == 0), stop=(hp == 3),
                    )
                fin = atp.tile([P, 512], bf16, tag="fin")
                nc.vector.tensor_add(fin[:], ps[:], beta_b[:, n * 512 : (n + 1) * 512])
                nc.sync.dma_start(
                    rs_in[i * P : (i + 1) * P, n * 512 : (n + 1) * 512], fin[:]
                )

            def rs_chunk(c):
                r0, r1 = CHUNKS[c]
                if globals().get("_NO_COLLECTIVE"):
                    nc.sync.dma_start(
                        out_d.ap()[r0 // 2 : r1 // 2, :],
                        rs_in[r0 : r0 + (r1 - r0) // 2, :],
                    )
                    return
                nc.gpsimd.collective_compute(
                    "ReduceScatter", mybir.AluOpType.add,
                    replica_groups=[[0, 1], [2, 3], [4, 5], [6, 7]],
                    ins=[rs_in[r0:r1, :].opt()],
                    outs=[rs_out[r0 // 2 : r1 // 2, :].opt()],
                )

            def out_dma(c):
                # deferred to kernel end: these wait on RS completion, so they
                # must not sit ahead of compute-feeding DMAs in the sync queue
                if globals().get("_NO_COLLECTIVE"):
                    return
                r0, r1 = CHUNKS[c]
                nc.sync.dma_start(
                    out_d.ap()[r0 // 2 : r1 // 2, :],
                    rs_out[r0 // 2 : r1 // 2, :],
                )

            def norm_copies(mp, J, dd):
                # raw-evict psum values + denominators so the slots free fast;
                # denominators land in the step-wide dd row at offset 1024*mp
                ouA, ouB = ou_tiles[mp]
                tb = tmpb.tile([64, 512], bf16, tag="tb")
                nc.vector.tensor_copy(dd[:, 1024 * mp : 1024 * mp + 512], ouA[64:65, :])
                nc.vector.tensor_copy(outN[mp][J][0:64, :], ouA[0:64, :])
                nc.vector.tensor_copy(dd[:, 1024 * mp + 512 : 1024 * (mp + 1)], ouB[64:65, :])
                nc.vector.tensor_copy(tb[:], ouB[0:64, :])
                nc.sync.dma_start(outN[mp][J][64:128, :], tb[:])

            def norm_finish(J, dd, mps):
                # one DRAM repack ([1,.] -> [128,.]) for all pending head
                # pairs so the DVE iterative reciprocal runs on all lanes,
                # then broadcast + scale each outN block.  outN is only read
                # by the (one-step-later) projection, so this chain is off
                # the attention critical path.
                n = 1024 * len(mps)
                off = 1024 * mps[0]
                p0, p1 = off // 32, (off + n) // 32  # partition range in the packed view
                dramD = dram2.tile([1, 4096], bf16, tag="dramD", name="dramD")
                nc.sync.dma_start(dramD[:, off : off + n], dd[:, off : off + n])
                dPack = recip.tile([P, 32], bf16, tag="dPack")
                pk = dramD[:].rearrange("a (p c) -> (a p) c", p=P)
                nc.sync.dma_start(dPack[p0:p1, :], pk[p0:p1, :])
                with nc.allow_low_precision(reason="bf16 softmax denominators; 0.4% rel is within tolerance"):
                    nc.vector.reciprocal(dPack[p0:p1, :], dPack[p0:p1, :])
                nc.sync.dma_start(pk[p0:p1, :], dPack[p0:p1, :])
                nc.sync.dma_start(dd[:, off : off + n], dramD[:, off : off + n])
                for mp in mps:
                    # full-width broadcasts: head A uses rows 0:64 of bc,
                    # head B rows 64:128 of bcB
                    bc = bcast.tile([64, 512], bf16, tag="bc")
                    nc.gpsimd.partition_broadcast(bc[:, :], dd[:, 1024 * mp : 1024 * mp + 512], channels=64)
                    bcB = bcast.tile([P, 512], bf16, tag="bcB")
                    nc.gpsimd.partition_broadcast(bcB[:, :], dd[:, 1024 * mp + 512 : 1024 * (mp + 1)], channels=P)
                    nc.vector.tensor_mul(outN[mp][J][0:64, :], outN[mp][J][0:64, :], bc[:, :])
                    nc.vector.tensor_mul(outN[mp][J][64:128, :], outN[mp][J][64:128, :], bcB[64:128, :])

            # ---- startup: t-quarter 0 prep + qk projection for column block 0
            xi0 = x_loads(0)
            for k2 in range(4):
                nc.sync.dma_start(
                    wv_t[k2][:].rearrange("p (b c) -> p b c", b=2),
                    wv_d.ap()[k2 * 2 * P : (k2 + 1) * 2 * P, :].rearrange(
                        "(b p) c -> p b c", p=P
                    ),
                )
            for hp in range(4):
                nc.sync.dma_start(
                    wproj_t[hp][:], wproj_d.ap()[hp * P : (hp + 1) * P, :]
                )
            _load_small_consts()
            for k in range(8):
                nc.sync.dma_start(wq8[k][:], wqk_d.ap()[k * P : (k + 1) * P, :])
            for k in range(8):
                trans_group(0, k, xi0)
            for il in range(4):
                vproj_group(0, il)
            for m in range(8):
                qkproj_group(m, 0)

            ou_tiles = {}
            for s in range(4):  # step = query block J = qk column block n
                # next t-quarter prep + qkproj fillers go in the first part of
                # the step; outproj fillers (which wait on the previous step's
                # normalization chains) go after 5/8, followed by the RS chunk.
                nj = 4 * s + 4
                total_iters = 4 * nj
                fillA = []
                if s < 3:
                    xi_n = x_loads(s + 1)
                    fillA += [(trans_group, (s + 1, k, xi_n)) for k in range(8)]
                    fillA += [(qkproj_group, (m, s + 1)) for m in range(8)]
                    fillA += [(vproj_group, (s + 1, il)) for il in range(4)]
                fillB = [(outproj_group, (s - 1, g)) for g in range(8)] if s > 0 else []
                if s > 0:
                    fillB.append((rs_chunk, (s - 1,)))
                bstart = (total_iters * 5) // 8
                strideA = max(1, bstart // max(1, len(fillA)))
                strideB = max(1, (total_iters - bstart) // max(1, len(fillB)))
                it = 0
                Js = slice(s * 512, (s + 1) * 512)
                dd = recip.tile([1, 4096], bf16, tag="dd")
                for mp in range(4):
                    qs, ks = qkT[mp], qkT[4 + mp]
                    ouA = oups.tile([65, 512], f32, tag="ouA")
                    ouB = oups.tile([65, 512], f32, tag="ouB")
                    ou_tiles[mp] = (ouA, ouB)
                    pending = None
                    for j in range(nj):
                        sT = stps.tile([P, 1024], f32, tag="sT")
                        js = slice(j * P, (j + 1) * P)
                        nc.tensor.matmul(
                            sT[:, 0:512],
                            ks[0:64, js], qs[0:64, Js],
                            start=True, stop=True, tile_position=(0, 0),
                        )
                        nc.tensor.matmul(
                            sT[:, 512:1024],
                            ks[64:128, js], qs[64:128, Js],
                            start=True, stop=True, tile_position=(64, 0),
                        )
                        at = atp.tile([P, 1024], bf16, tag="at")
                        i = j - 4 * s
                        c0 = 128 * i if i > 0 else 0
                        if i > 0:
                            src_v = sT[:].rearrange("p (h c) -> p h c", h=2)
                            dst_v = at[:].rearrange("p (h c) -> p h c", h=2)
                            nc.scalar.activation(
                                dst_v[:, :, c0:512], src_v[:, :, c0:512],
                                EXP, bias=0.0, scale=0.125,
                            )
                        else:
                            nc.scalar.activation(at[:], sT[:], EXP, bias=0.0, scale=0.125)
                        if i >= 0:
                            # diagonal block: one strided multiply applies the
                            # triangle to both heads' [c0,c0+128)
                            atv = at[:].rearrange("p (h c) -> p h c", h=2)
                            mkv = mask_tri2[:].rearrange("p (h c) -> p h c", h=2)
                            nc.vector.tensor_mul(
                                atv[:, :, c0 : c0 + 128],
                                atv[:, :, c0 : c0 + 128],
                                mkv[:, :, :],
                            )
                        # software-pipelined AV: one iteration behind scores
                        if pending is not None:
                            _emit_av(nc, mp, s, nj, pending, ou_tiles[mp], vv, atv_hist)
                        atv_hist[j] = at
                        pending = j
                        it += 1
                        if it <= bstart:
                            if fillA and it % strideA == 0:
                                fn, args = fillA.pop(0)
                                fn(*args)
                        else:
                            if fillB and (it - bstart) % strideB == 0:
                                fn, args = fillB.pop(0)
                                fn(*args)
                    _emit_av(nc, mp, s, nj, pending, ou_tiles[mp], vv, atv_hist)
                    norm_copies(mp, s, dd)
                    if s == 3:
                        # last step: per-mp chains so only mp=3's is tail
                        norm_finish(s, dd, [mp])
                if s < 3:
                    norm_finish(s, dd, [0, 1, 2, 3])
                for fn, args in fillA + fillB:
                    fn(*args)
            # final projection block + last chunk
            for g in range(8):
                outproj_group(3, g)
            rs_chunk(3)
            for c in range(4):
                out_dma(c)
            _dbg(nc, "qkT0", qkT[0][:])
            _dbg(nc, "rs_in", rs_in[:])


atv_hist = {}


def _emit_av(nc, mp, s, nj, j, ou, vv, hist):
    ouA, ouB = ou
    at = hist[j]
    i = j - 4 * s
    c0 = 128 * i if i > 0 else 0
    nc.tensor.matmul(
        ouA[:, c0:512], vv[j][:, 130 * mp : 130 * mp + 65],
        at[:, c0:512],
        start=(j == 0), stop=(j == nj - 1),
        skip_group_check=(i > 0 or j == nj - 1),
    )
    nc.tensor.matmul(
        ouB[:, c0:512], vv[j][:, 130 * mp + 65 : 130 * mp + 130],
        at[:, 512 + c0 : 1024],
        start=(j == 0), stop=(j == nj - 1),
        skip_group_check=(i > 0 or j == nj - 1),
    )


def _build():
    if "nc" in _CACHE:
        return _CACHE["nc"]
    nc = bacc.Bacc("TRN2", target_bir_lowering=False, debug=False, num_devices=NCORES)
    x_d = nc.dram_tensor("x", [T, D], bf16, kind="ExternalInput")
    wqk_d = nc.dram_tensor("w_qk", [D, 1024], bf16, kind="ExternalInput")
    wv_d = nc.dram_tensor("w_v", [D, 512], bf16, kind="ExternalInput")
    bqk_d = nc.dram_tensor("b_qk", [8, P, 1], f32, kind="ExternalInput")
    wproj_d = nc.dram_tensor("w_proj", [512, D], bf16, kind="ExternalInput")
    beta_d = nc.dram_tensor("beta", [1, D], bf16, kind="ExternalInput")
    out_d = nc.dram_tensor("out", [T // 2, D], bf16, kind="ExternalOutput")
    with tile.TileContext(nc) as tc:
        _emit(nc, tc, x_d, wqk_d, wv_d, bqk_d, wproj_d, beta_d, out_d)
    nc.compile()
    _CACHE["nc"] = nc
    return nc


def make_in_maps(x, w_qkv, b_qkv, w_proj, b_proj):
    x = np.asarray(x, np.float32)
    w_qkv = np.asarray(w_qkv, np.float32)
    b_qkv = np.asarray(b_qkv, np.float32)
    w_proj = np.asarray(w_proj, np.float32)
    b_proj = np.asarray(b_proj, np.float32)
    in_maps = []
    for c in range(NCORES):
        b, g = c // 2, c % 2
        qcols = slice(g * 512, (g + 1) * 512)
        kcols = slice(D + g * 512, D + (g + 1) * 512)
        vcols = slice(2 * D + g * 512, 2 * D + (g + 1) * 512)
        w_qk = np.concatenate([w_qkv[:, qcols], w_qkv[:, kcols]], axis=1)
        b_qk = np.concatenate([b_qkv[qcols], b_qkv[kcols]])
        wp = np.ascontiguousarray(w_proj[g * 512 : (g + 1) * 512, :])
        beta = wp.T @ b_qkv[vcols]
        if g == 0:
            beta = beta + b_proj
        in_maps.append({
            "x": np.ascontiguousarray(x[b]).astype(ml_dtypes.bfloat16),
            "w_qk": np.ascontiguousarray(w_qk).astype(ml_dtypes.bfloat16),
            "w_v": np.ascontiguousarray(w_qkv[:, vcols]).astype(ml_dtypes.bfloat16),
            "b_qk": b_qk.reshape(8, P, 1),
            "w_proj": wp.astype(ml_dtypes.bfloat16),
            "beta": beta.reshape(1, D).astype(ml_dtypes.bfloat16),
        })
    return in_maps


def kernel(x, w_qkv, b_qkv, w_proj, b_proj, trace=False, **run_kwargs):
    global LAST_RESULTS
    nc = _build()
    in_maps = make_in_maps(x, w_qkv, b_qkv, w_proj, b_proj)
    res = run_bass_kernel_spmd(
        nc, in_maps, core_ids=list(range(NCORES)), trace=trace, **run_kwargs
    )
    LAST_RESULTS = res
    out = np.empty((B, T, D), np.float32)
    for b in range(B):
        ev = np.asarray(res.results[2 * b]["out"], np.float32)
        od = np.asarray(res.results[2 * b + 1]["out"], np.float32)
        for r0, r1 in CHUNKS:
            mid = (r0 + r1) // 2
            out[b, r0:mid] = ev[r0 // 2 : r1 // 2]
            out[b, mid:r1] = od[r0 // 2 : r1 // 2]
    return out


# revision 27
# speedup vs baseline: 1.0289x; 1.0289x over previous
"""Causal self-attention Bass kernel for 8 trn2 NeuronCores.

Problem: B=4, T=2048, D=1024, H=16 causal self-attention (qkv proj + attn + out proj).

Sharding: core c = 2*b + g handles batch b (=c//2) and head-group g (=c%2, 8 heads).
Per core:
  - qkv projection column-shard: q,k,v columns for its 8 heads only.
  - flash-style attention in transposed-score layout sT[tk, tq]; softmax denominator
    via an extra ones-column in the AV matmul (row 64 of the [65, 512] psum output).
  - output projection row-shard (w_proj rows for its head dims) -> partial [T, D].
  - pairwise ReduceScatter {2b, 2b+1} sums the two head-group partials; chunked
    over 4 query blocks so the collective overlaps the attention/projection
    stream.  Even core ends with global rows {512J..512J+256}, odd core with
    {512J+256..512J+512}; host reassembles.

Pipeline: phase 2 runs query-block-outer (J = 0..3).  Step J computes attention
for all 4 head-pairs at query block J, while interleaving (as PE filler work
between ACT-gated attention iterations) the qk-projection for column block J+1
and the output projection + ReduceScatter chunk for block J-1.  The AV matmul is
software-pipelined one iteration behind the score matmul so the PE never waits
on the exp/mask chain.

Precision: bf16 operands everywhere on the matmul paths (1 cyc/row streaming,
FWL weight loads); psum accumulation stays f32.  The ReduceScatter runs in bf16;
host converts the output to f32.  b_v is folded into beta = b_proj(once per
pair) + w_proj_shard.T @ b_v_shard since softmax rows sum to 1.

Diagonal blocks: partial exp (cols >= 128*i only) and ONE strided triangle-mask
multiply covering both heads; AV matmuls read only the valid column range, so no
zero-fill of the masked region is needed.
"""

from contextlib import ExitStack

import ml_dtypes
import numpy as np

import concourse.bass as bass
import concourse.mybir as mybir
import concourse.tile as tile
from concourse import bacc
from concourse.bass_utils import run_bass_kernel_spmd

B, T, D, H = 4, 2048, 1024, 16
HD = D // H  # 64
NCORES = 8
P = 128
f32 = mybir.dt.float32
f32r = mybir.dt.float32r
bf16 = mybir.dt.bfloat16
EXP = mybir.ActivationFunctionType.Exp

_CACHE = {}
LAST_RESULTS = None
_DEBUG_SINK = None

# ReduceScatter chunk boundaries (input row ranges of the [T, D] partial sums)
CHUNKS = [(0, 512), (512, 1024), (1024, 1536), (1536, 2048)]


def _dbg(nc, name, ap):
    if _DEBUG_SINK is not None and name in _DEBUG_SINK:
        nc.sync.dma_start(_DEBUG_SINK[name].ap(), ap)


def _emit(nc, tc, x_d, wqk_d, wv_d, bqk_d, wproj_d, beta_d, out_d):
    with ExitStack() as ctx:
        # ---------------- constants / persistent tiles ----------------
        const = ctx.enter_context(tc.tile_pool(name="const", bufs=1))
        bootc = ctx.enter_context(tc.tile_pool(name="boot", bufs=1))
        ident_f = bootc.tile([P, P], bf16, tag="ident_f")
        nc.gpsimd.memset(ident_f[:], 0.0)
        nc.gpsimd.affine_select(
            out=ident_f[:], in_=ident_f[:],
            compare_op=mybir.AluOpType.not_equal, fill=1.0,
            base=0, pattern=[[-1, P]], channel_multiplier=1,
        )
        # triangle mask [128,256] = two copies of (keep iff f >= p)
        mask_tri2 = const.tile([P, 2 * P], bf16, tag="mask_tri2")
        nc.gpsimd.memset(mask_tri2[:], 1.0)
        for h0 in (0, P):
            nc.gpsimd.affine_select(
                out=mask_tri2[:, h0 : h0 + P], in_=mask_tri2[:, h0 : h0 + P],
                compare_op=mybir.AluOpType.is_ge, fill=0.0,
                base=0, pattern=[[1, P]], channel_multiplier=-1,
            )
        bq8 = const.tile([P, 8], f32, tag="bq8")
        beta_b = const.tile([P, D], bf16, tag="beta_b")

        def _load_small_consts():
            nc.sync.dma_start(bq8[:], bqk_d.ap().rearrange("m p one -> p (m one)"))
            nc.sync.dma_start(beta_b[0:1, :], beta_d.ap())
            nc.gpsimd.partition_broadcast(beta_b[:], beta_b[0:1, :], channels=P)
        # weight pools reserved here; DMAs are emitted after phase 1 starts
        # so the x loads win the DMA queue.
        wpp = ctx.enter_context(tc.tile_pool(name="wpp", bufs=1))
        wproj_t = [wpp.tile([P, D], bf16, tag=f"wp{hp}", name=f"wp{hp}") for hp in range(4)]
        wqkp = ctx.enter_context(tc.tile_pool(name="wqk", bufs=1))
        # one [P, 1024] tile per k-chunk holding all 8 m column blocks
        wq8 = [wqkp.tile([P, 1024], bf16, tag=f"wq8k{k}", name=f"wq8k{k}") for k in range(8)]
        _dbg(nc, "beta_b", beta_b[:])

        # persistent activations
        xt_pool = ctx.enter_context(tc.tile_pool(name="xt", bufs=1))
        xT = [xt_pool.tile([P, T], bf16, tag=f"xT{k}", name=f"xT{k}") for k in range(8)]
        vv_pool = ctx.enter_context(tc.tile_pool(name="vv", bufs=1))
        vv = [vv_pool.tile([P, 520], bf16, tag=f"vv{i}", name=f"vv{i}") for i in range(16)]
        on_pool = ctx.enter_context(tc.tile_pool(name="outn", bufs=1))
        outN = [[on_pool.tile([P, 512], bf16, tag=f"outN{mp}J{J}", name=f"outN{mp}J{J}")
                 for J in range(4)] for mp in range(4)]
        ones8 = const.tile([P, 8], bf16, tag="ones8")
        nc.vector.memset(ones8[:], 1.0)
        ones_src = ones8[:].rearrange("p (mp h one) -> p mp h one", mp=4, h=2)
        for i in range(16):
            dst = vv[i][:].rearrange("p (mp h d) -> p mp h d", mp=4, h=2)
            nc.vector.tensor_copy(dst[:, :, :, 64:65], ones_src[:, :, :, :])

        dram = ctx.enter_context(tc.tile_pool(name="dram", bufs=1, space="DRAM"))
        rs_in = dram.tile([T, D], bf16)
        rs_out = dram.tile([T // 2, D], bf16)
        dram2 = ctx.enter_context(tc.tile_pool(name="dram2", bufs=2, space="DRAM"))

        # ------- unified pipeline: x/v prep is filler work inside the steps -------
        with ExitStack() as p2:
            xload = p2.enter_context(tc.tile_pool(name="xload", bufs=3))
            wvp = p2.enter_context(tc.tile_pool(name="wv", bufs=1))
            qkt_pool = p2.enter_context(tc.tile_pool(name="qkt", bufs=1))
            qkT = [qkt_pool.tile([P, T], bf16, tag=f"qkT{m}", name=f"qkT{m}") for m in range(8)]
            atp = p2.enter_context(tc.tile_pool(name="atp", bufs=3))
            recip = p2.enter_context(tc.tile_pool(name="recip", bufs=2))
            bcast = p2.enter_context(tc.tile_pool(name="bcast", bufs=2))
            tmpb = p2.enter_context(tc.tile_pool(name="tmpb", bufs=4))
            qkps = p2.enter_context(tc.tile_pool(name="qkps", bufs=2, space="PSUM"))
            stps = p2.enter_context(tc.tile_pool(name="stps", bufs=2, space="PSUM"))
            oups = p2.enter_context(tc.tile_pool(name="oups", bufs=1, space="PSUM"))
            # [P, 1024] tiles, each holding two k-chunks side by side
            wv_t = [wvp.tile([P, 1024], bf16, tag=f"wvt{k2}", name=f"wvt{k2}") for k2 in range(4)]

            def x_loads(qq):
                # one [P, 2048] load covers two t-tiles (rows r0..r0+256)
                xi2 = []
                for ii in range(2):
                    xt_ = xload.tile([P, 2 * D], bf16, tag="x")
                    r0 = (qq * 4 + 2 * ii) * P
                    nc.sync.dma_start(
                        xt_[:].rearrange("p (b c) -> p b c", b=2),
                        x_d.ap()[r0 : r0 + 2 * P, :].rearrange("(b p) c -> p b c", p=P),
                    )
                    xi2.append(xt_)
                return [
                    xi2[ii // 2][:, (ii % 2) * D : (ii % 2 + 1) * D]
                    for ii in range(4)
                ]

            def trans_group(qq, k, xi):
                # transpose one k-chunk of a t-quarter into xT (evict on DVE)
                tp = qkps.tile([P, 512], bf16, tag="qkp", name="tp")
                for ii in range(4):
                    nc.tensor.transpose(
                        tp[:, ii * P : (ii + 1) * P],
                        xi[ii][:, k * P : (k + 1) * P],
                        ident_f[:],
                    )
                nc.vector.tensor_copy(xT[k][:, qq * 512 : (qq + 1) * 512], tp[:])

            def vproj_group(qq, il):
                i = qq * 4 + il
                ps = qkps.tile([P, 512], f32, tag="qkp", name="vp")
                for k in range(8):
                    nc.tensor.matmul(
                        ps[:],
                        xT[k][:, i * P : (i + 1) * P],
                        wv_t[k // 2][:, (k % 2) * 512 : (k % 2 + 1) * 512],
                        start=(k == 0), stop=(k == 7),
                    )
                # strided evict: psum [p, (mp h d)] d=64 -> vv [p, (mp h d65)]
                src = ps[:].rearrange("p (mp h d) -> p mp h d", mp=4, h=2)
                dst = vv[i][:].rearrange("p (mp h d) -> p mp h d", mp=4, h=2)
                nc.vector.tensor_copy(dst[:, :, :, 0:64], src[:, :, :, :])

            def qkproj_group(m, n):
                ps = qkps.tile([P, 512], f32, tag="qkp")
                for k in range(8):
                    nc.tensor.matmul(
                        ps[:], wq8[k][:, m * P : (m + 1) * P],
                        xT[k][:, n * 512 : (n + 1) * 512],
                        start=(k == 0), stop=(k == 7),
                    )
                nc.vector.tensor_scalar_add(
                    qkT[m][:, n * 512 : (n + 1) * 512], ps[:], bq8[:, m : m + 1]
                )

            def outproj_group(J, g):
                i = 4 * J + g // 2
                n = g % 2
                ps = qkps.tile([P, 512], f32, tag="qkp", name="fp")
                for hp in range(4):
                    nc.tensor.matmul(
                        ps[:],
                        outN[hp][J][:, (i % 4) * P : (i % 4 + 1) * P],
                        wproj_t[hp][:, n * 512 : (n + 1) * 512],
                        start=(hp == 0), stop=(hp == 3),
                    )
                fin = atp.tile([P, 512], bf16, tag="fin")
                nc.vector.tensor_add(fin[:], ps[:], beta_b[:, n * 512 : (n + 1) * 512])
                nc.sync.dma_start(
                    rs_in[i * P : (i + 1) * P, n * 512 : (n + 1) * 512], fin[:]
                )

            def rs_chunk(c):
                r0, r1 = CHUNKS[c]
                if globals().get("_NO_COLLECTIVE"):
                    nc.sync.dma_start(
                        out_d.ap()[r0 // 2 : r1 // 2, :],
                        rs_in[r0 : r0 + (r1 - r0) // 2, :],
                    )
                    return
                nc.gpsimd.collective_compute(
                    "ReduceScatter", mybir.AluOpType.add,
                    replica_groups=[[0, 1], [2, 3], [4, 5], [6, 7]],
                    ins=[rs_in[r0:r1, :].opt()],
                    outs=[rs_out[r0 // 2 : r1 // 2, :].opt()],
                )

            def out_dma(c):
                # deferred to kernel end: these wait on RS completion, so they
                # must not sit ahead of compute-feeding DMAs in the sync queue
                if globals().get("_NO_COLLECTIVE"):
                    return
                r0, r1 = CHUNKS[c]
                nc.sync.dma_start(
                    out_d.ap()[r0 // 2 : r1 // 2, :],
                    rs_out[r0 // 2 : r1 // 2, :],
                )

            def norm_copies(mp, J, dd):
                # raw-evict psum values + denominators so the slots free fast;
                # denominators land in the step-wide dd row at offset 1024*mp
                ouA, ouB = ou_tiles[mp]
                tb = tmpb.tile([64, 512], bf16, tag="tb")
                nc.vector.tensor_copy(dd[:, 1024 * mp : 1024 * mp + 512], ouA[64:65, :])
                nc.vector.tensor_copy(outN[mp][J][0:64, :], ouA[0:64, :])
                nc.vector.tensor_copy(dd[:, 1024 * mp + 512 : 1024 * (mp + 1)], ouB[64:65, :])
                nc.vector.tensor_copy(tb[:], ouB[0:64, :])
                nc.sync.dma_start(outN[mp][J][64:128, :], tb[:])

            def norm_finish(J, dd, mps):
                # one DRAM repack ([1,.] -> [128,.]) for all pending head
                # pairs so the DVE iterative reciprocal runs on all lanes,
                # then broadcast + scale each outN block.  outN is only read
                # by the (one-step-later) projection, so this chain is off
                # the attention critical path.
                n = 1024 * len(mps)
                off = 1024 * mps[0]
                p0, p1 = off // 32, (off + n) // 32  # partition range in the packed view
                dramD = dram2.tile([1, 4096], bf16, tag="dramD", name="dramD")
                nc.sync.dma_start(dramD[:, off : off + n], dd[:, off : off + n])
                dPack = recip.tile([P, 32], bf16, tag="dPack")
                pk = dramD[:].rearrange("a (p c) -> (a p) c", p=P)
                nc.sync.dma_start(dPack[p0:p1, :], pk[p0:p1, :])
                with nc.allow_low_precision(reason="bf16 softmax denominators; 0.4% rel is within tolerance"):
                    nc.vector.reciprocal(dPack[p0:p1, :], dPack[p0:p1, :])
                nc.sync.dma_start(pk[p0:p1, :], dPack[p0:p1, :])
                nc.sync.dma_start(dd[:, off : off + n], dramD[:, off : off + n])
                for mp in mps:
                    # full-width broadcasts: head A uses rows 0:64 of bc,
                    # head B rows 64:128 of bcB
                    bc = bcast.tile([64, 512], bf16, tag="bc")
                    nc.gpsimd.partition_broadcast(bc[:, :], dd[:, 1024 * mp : 1024 * mp + 512], channels=64)
                    bcB = bcast.tile([P, 512], bf16, tag="bcB")
                    nc.gpsimd.partition_broadcast(bcB[:, :], dd[:, 1024 * mp + 512 : 1024 * (mp + 1)], channels=P)
                    nc.vector.tensor_mul(outN[mp][J][0:64, :], outN[mp][J][0:64, :], bc[:, :])
                    nc.vector.tensor_mul(outN[mp][J][64:128, :], outN[mp][J][64:128, :], bcB[64:128, :])

            # ---- startup: t-quarter 0 prep + qk projection for column block 0
            xi0 = x_loads(0)
            for k2 in range(4):
                nc.sync.dma_start(
                    wv_t[k2][:].rearrange("p (b c) -> p b c", b=2),
                    wv_d.ap()[k2 * 2 * P : (k2 + 1) * 2 * P, :].rearrange(
                        "(b p) c -> p b c", p=P
                    ),
                )
            for hp in range(4):
                nc.sync.dma_start(
                    wproj_t[hp][:], wproj_d.ap()[hp * P : (hp + 1) * P, :]
                )
            _load_small_consts()
            for k in range(8):
                nc.sync.dma_start(wq8[k][:], wqk_d.ap()[k * P : (k + 1) * P, :])
            for k in range(8):
                trans_group(0, k, xi0)
            for m in range(8):
                qkproj_group(m, 0)
            for il in range(4):
                vproj_group(0, il)

            ou_tiles = {}
            for s in range(4):  # step = query block J = qk column block n
                # next t-quarter prep + qkproj fillers go in the first part of
                # the step; outproj fillers (which wait on the previous step's
                # normalization chains) go after 5/8, followed by the RS chunk.
                nj = 4 * s + 4
                total_iters = 4 * nj
                fillA = []
                if s < 3:
                    xi_n = x_loads(s + 1)
                    fillA += [(trans_group, (s + 1, k, xi_n)) for k in range(8)]
                    fillA += [(qkproj_group, (m, s + 1)) for m in range(8)]
                    fillA += [(vproj_group, (s + 1, il)) for il in range(4)]
                fillB = [(outproj_group, (s - 1, g)) for g in range(8)] if s > 0 else []
                if s > 0:
                    fillB.append((rs_chunk, (s - 1,)))
                bstart = (total_iters * 5) // 8
                strideA = max(1, bstart // max(1, len(fillA)))
                strideB = max(1, (total_iters - bstart) // max(1, len(fillB)))
                it = 0
                Js = slice(s * 512, (s + 1) * 512)
                dd = recip.tile([1, 4096], bf16, tag="dd")
                for mp in range(4):
                    qs, ks = qkT[mp], qkT[4 + mp]
                    ouA = oups.tile([65, 512], f32, tag="ouA")
                    ouB = oups.tile([65, 512], f32, tag="ouB")
                    ou_tiles[mp] = (ouA, ouB)
                    pending = None
                    for j in range(nj):
                        sT = stps.tile([P, 1024], f32, tag="sT")
                        js = slice(j * P, (j + 1) * P)
                        i = j - 4 * s
                        c0 = 128 * i if i > 0 else 0
                        Jc = slice(s * 512 + c0, (s + 1) * 512)
                        nc.tensor.matmul(
                            sT[:, c0:512],
                            ks[0:64, js], qs[0:64, Jc],
                            start=True, stop=True, tile_position=(0, 0),
                            skip_group_check=(i > 0),
                        )
                        nc.tensor.matmul(
                            sT[:, 512 + c0 : 1024],
                            ks[64:128, js], qs[64:128, Jc],
                            start=True, stop=True, tile_position=(64, 0),
                            skip_group_check=(i > 0),
                        )
                        at = atp.tile([P, 1024], bf16, tag="at")
                        if i > 0:
                            src_v = sT[:].rearrange("p (h c) -> p h c", h=2)
                            dst_v = at[:].rearrange("p (h c) -> p h c", h=2)
                            nc.scalar.activation(
                                dst_v[:, :, c0:512], src_v[:, :, c0:512],
                                EXP, bias=0.0, scale=0.125,
                            )
                        else:
                            nc.scalar.activation(at[:], sT[:], EXP, bias=0.0, scale=0.125)
                        if i >= 0:
                            # diagonal block: one strided multiply applies the
                            # triangle to both heads' [c0,c0+128)
                            atv = at[:].rearrange("p (h c) -> p h c", h=2)
                            mkv = mask_tri2[:].rearrange("p (h c) -> p h c", h=2)
                            nc.vector.tensor_mul(
                                atv[:, :, c0 : c0 + 128],
                                atv[:, :, c0 : c0 + 128],
                                mkv[:, :, :],
                            )
                        # software-pipelined AV: one iteration behind scores
                        if pending is not None:
                            _emit_av(nc, mp, s, nj, pending, ou_tiles[mp], vv, atv_hist)
                        atv_hist[j] = at
                        pending = j
                        it += 1
                        if it <= bstart:
                            if fillA and it % strideA == 0:
                                fn, args = fillA.pop(0)
                                fn(*args)
                        else:
                            if fillB and (it - bstart) % strideB == 0:
                                fn, args = fillB.pop(0)
                                fn(*args)
                    _emit_av(nc, mp, s, nj, pending, ou_tiles[mp], vv, atv_hist)
                    norm_copies(mp, s, dd)
                    if s == 3 and mp == 2:
                        # last step: batch the first three chains; mp=3 gets
                        # its own short chain so only that one is tail
                        norm_finish(s, dd, [0, 1, 2])
                    elif s == 3 and mp == 3:
                        norm_finish(s, dd, [3])
                if s < 3:
                    norm_finish(s, dd, [0, 1, 2, 3])
                for fn, args in fillA + fillB:
                    fn(*args)
            # final projection block + last chunk
            for g in range(8):
                outproj_group(3, g)
            rs_chunk(3)
            for c in range(4):
                out_dma(c)
            _dbg(nc, "qkT0", qkT[0][:])
            _dbg(nc, "rs_in", rs_in[:])


atv_hist = {}


def _emit_av(nc, mp, s, nj, j, ou, vv, hist):
    ouA, ouB = ou
    at = hist[j]
    i = j - 4 * s
    c0 = 128 * i if i > 0 else 0
    nc.tensor.matmul(
        ouA[:, c0:512], vv[j][:, 130 * mp : 130 * mp + 65],
        at[:, c0:512],
        start=(j == 0), stop=(j == nj - 1),
        skip_group_check=(i > 0 or j == nj - 1),
    )
    nc.tensor.matmul(
        ouB[:, c0:512], vv[j][:, 130 * mp + 65 : 130 * mp + 130],
        at[:, 512 + c0 : 1024],
        start=(j == 0), stop=(j == nj - 1),
        skip_group_check=(i > 0 or j == nj - 1),
    )


def _build():
    if "nc" in _CACHE:
        return _CACHE["nc"]
    nc = bacc.Bacc("TRN2", target_bir_lowering=False, debug=False, num_devices=NCORES)
    x_d = nc.dram_tensor("x", [T, D], bf16, kind="ExternalInput")
    wqk_d = nc.dram_tensor("w_qk", [D, 1024], bf16, kind="ExternalInput")
    wv_d = nc.dram_tensor("w_v", [D, 512], bf16, kind="ExternalInput")
    bqk_d = nc.dram_tensor("b_qk", [8, P, 1], f32, kind="ExternalInput")
    wproj_d = nc.dram_tensor("w_proj", [512, D], bf16, kind="ExternalInput")
    beta_d = nc.dram_tensor("beta", [1, D], bf16, kind="ExternalInput")
    out_d = nc.dram_tensor("out", [T // 2, D], bf16, kind="ExternalOutput")
    with tile.TileContext(nc) as tc:
        _emit(nc, tc, x_d, wqk_d, wv_d, bqk_d, wproj_d, beta_d, out_d)
    nc.compile()
    _CACHE["nc"] = nc
    return nc


def make_in_maps(x, w_qkv, b_qkv, w_proj, b_proj):
    x = np.asarray(x, np.float32)
    w_qkv = np.asarray(w_qkv, np.float32)
    b_qkv = np.asarray(b_qkv, np.float32)
    w_proj = np.asarray(w_proj, np.float32)
    b_proj = np.asarray(b_proj, np.float32)
    in_maps = []
    for c in range(NCORES):
        b, g = c // 2, c % 2
        qcols = slice(g * 512, (g + 1) * 512)
        kcols = slice(D + g * 512, D + (g + 1) * 512)
        vcols = slice(2 * D + g * 512, 2 * D + (g + 1) * 512)
        w_qk = np.concatenate([w_qkv[:, qcols], w_qkv[:, kcols]], axis=1)
        b_qk = np.concatenate([b_qkv[qcols], b_qkv[kcols]])
        wp = np.ascontiguousarray(w_proj[g * 512 : (g + 1) * 512, :])
        beta = wp.T @ b_qkv[vcols]
        if g == 0:
            beta = beta + b_proj
        in_maps.append({
            "x": np.ascontiguousarray(x[b]).astype(ml_dtypes.bfloat16),
            "w_qk": np.ascontiguousarray(w_qk).astype(ml_dtypes.bfloat16),
            "w_v": np.ascontiguousarray(w_qkv[:, vcols]).astype(ml_dtypes.bfloat16),
            "b_qk": b_qk.reshape(8, P, 1),
            "w_proj": wp.astype(ml_dtypes.bfloat16),
            "beta": beta.reshape(1, D).astype(ml_dtypes.bfloat16),
        })
    return in_maps


def kernel(x, w_qkv, b_qkv, w_proj, b_proj, trace=False, **run_kwargs):
    global LAST_RESULTS
    nc = _build()
    in_maps = make_in_maps(x, w_qkv, b_qkv, w_proj, b_proj)
    res = run_bass_kernel_spmd(
        nc, in_maps, core_ids=list(range(NCORES)), trace=trace, **run_kwargs
    )
    LAST_RESULTS = res
    out = np.empty((B, T, D), np.float32)
    for b in range(B):
        ev = np.asarray(res.results[2 * b]["out"], np.float32)
        od = np.asarray(res.results[2 * b + 1]["out"], np.float32)
        for r0, r1 in CHUNKS:
            mid = (r0 + r1) // 2
            out[b, r0:mid] = ev[r0 // 2 : r1 // 2]
            out[b, mid:r1] = od[r0 // 2 : r1 // 2]
    return out
